# revision 24
# baseline (speedup 1.0000x reference)
"""GroupNorm + single-head self-attention + residual block on 8 trn2 cores.

Reference computation (per batch item b of 64):
    xn = GroupNorm32(x[b]) * gn_w + gn_b          # x[b]: [C=128, HW=1024]
    t  = xn^T                                     # [S=1024, C=128]
    q, k, v = t@wq^T+bq, t@wk^T+bk, t@wv^T+bv
    att = softmax(q k^T / sqrt(512))
    out[b] = (att v) @ wo^T + bo  (as [C, HW])  + x[b]

Sharding: pure data parallel, 8 batch items per core, params replicated.

Kernel layout choices (per batch item, all on-chip):
  - channels live on SBUF partitions; sequence S=1024 on the free dim
  - attention scores computed TRANSPOSED: attT[t, s] = kT^T qT, so the
    P^T needed by the output matmul is produced directly and no PE
    transposes are needed
  - softmax skips the max-subtraction (logits are provably in [-2, 2]);
    exp via ScalarE; row sums via an all-ones stationary matmul that
    accumulates exp blocks into a partition-broadcast PSUM tile
  - wv and wo are fused on the host: W = xn^T @ (wo@wv)^T, so attention
    output directly accumulates o2^T = W^T exp without a second
    projection; v-bias folds into bo_eff = bo + wo@bv; normalization by
    the softmax sum commutes to the very end
  - matmul operands are bf16 (outputs accumulate in fp32 PSUM); the
    residual path, groupnorm stats and softmax normalization stay fp32
  - groupnorm stats are hoisted per 4-batch group (one Sqrt table load
    per group instead of per batch)
"""

import numpy as np

import concourse.bacc as bacc
import concourse.bass as bass
import concourse.tile as tile
from concourse import mybir
from concourse.bass import _add_dep_helper
from concourse.bass_utils import run_bass_kernel_spmd

f32 = mybir.dt.float32
f32r = mybir.dt.float32r
bf16 = mybir.dt.bfloat16
AX = mybir.AxisListType
AF = mybir.ActivationFunctionType
OP = mybir.AluOpType

N_CORES = 8
B, C, HW = 64, 128, 1024
BPC = B // N_CORES          # batch items per core
NBLK = HW // 128            # 8 key blocks of 128
GRP = 4                     # batches per groupnorm stats group
SCALE = 0.044194173824159216
EPS = 1e-6

_NC_CACHE = None


def _build_nc():
    nc = bacc.Bacc()

    x_d = nc.declare_dram_parameter("x", [BPC, C, HW], f32, isOutput=False)
    wq_d = nc.declare_dram_parameter("wq_t", [C, C], f32, isOutput=False)
    wk_d = nc.declare_dram_parameter("wk_t", [C, C], f32, isOutput=False)
    wvo_d = nc.declare_dram_parameter("wvo_t", [C, C], f32, isOutput=False)
    bq_d = nc.declare_dram_parameter("bq", [C, 1], f32, isOutput=False)
    bk_d = nc.declare_dram_parameter("bk", [C, 1], f32, isOutput=False)
    bo_d = nc.declare_dram_parameter("bo_eff", [C, 1], f32, isOutput=False)
    gw_d = nc.declare_dram_parameter("gn_w", [C, 1], f32, isOutput=False)
    gb_d = nc.declare_dram_parameter("gn_b", [C, 1], f32, isOutput=False)
    gmat_d = nc.declare_dram_parameter("gmat", [C, 32], f32r, isOutput=False)
    rmat_d = nc.declare_dram_parameter("rmat", [32, C], f32r, isOutput=False)
    out_d = nc.declare_dram_parameter("out", [BPC, C, HW], f32, isOutput=True)

    with tile.TileContext(nc) as tc:
        with (
            tc.tile_pool(name="const", bufs=1) as const,
            tc.tile_pool(name="xin", bufs=8) as xin,
            tc.tile_pool(name="xnp", bufs=2) as xnp,
            tc.tile_pool(name="qkw", bufs=2) as qkw,
            tc.tile_pool(name="expp", bufs=4) as expp,
            tc.tile_pool(name="epi", bufs=2) as epi,
            tc.tile_pool(name="small", bufs=4) as small,
            tc.tile_pool(name="gn", bufs=2) as gnp,
            tc.tile_pool(name="ps_att", bufs=2, space="PSUM") as ps_att,
            tc.tile_pool(name="ps_row", bufs=1, space="PSUM") as ps_row,
            tc.tile_pool(name="ps_o2", bufs=1, space="PSUM") as ps_o2,
        ):
            # ---- one-time constants ----
            stage = const.tile([C, C], f32, tag="stage_q")
            nc.gpsimd.dma_start(out=stage, in_=wq_d[:, :])
            wq_r = const.tile([C, C], f32r, tag="wq_r")
            nc.gpsimd.tensor_copy(out=wq_r, in_=stage)

            stage2 = const.tile([C, C], f32, tag="stage_k")
            nc.gpsimd.dma_start(out=stage2, in_=wk_d[:, :])
            wk_r = const.tile([C, C], f32r, tag="wk_r")
            nc.gpsimd.tensor_copy(out=wk_r, in_=stage2)

            stage3 = const.tile([C, C], f32, tag="stage_v")
            nc.gpsimd.dma_start(out=stage3, in_=wvo_d[:, :])
            wvo_r = const.tile([C, C], f32r, tag="wvo_r")
            nc.gpsimd.tensor_copy(out=wvo_r, in_=stage3)

            ones_s = const.tile([C, C], f32, tag="ones_s")
            nc.vector.memset(ones_s, 1.0)
            ones_r = const.tile([C, C], bf16, tag="ones_r")
            nc.gpsimd.tensor_copy(out=ones_r, in_=ones_s)

            gmat_s = const.tile([C, 32], f32r, tag="gmat_s")
            nc.sync.dma_start(out=gmat_s, in_=gmat_d[:, :])
            rmat_s = const.tile([32, C], f32r, tag="rmat_s")
            nc.sync.dma_start(out=rmat_s, in_=rmat_d[:, :])

            bq_c = const.tile([C, 1], f32, tag="bq_c")
            nc.gpsimd.dma_start(out=bq_c, in_=bq_d[:, :])
            bk_c = const.tile([C, 1], f32, tag="bk_c")
            nc.gpsimd.dma_start(out=bk_c, in_=bk_d[:, :])
            bo_c = const.tile([C, 1], f32, tag="bo_c")
            nc.gpsimd.dma_start(out=bo_c, in_=bo_d[:, :])
            gw_c = const.tile([C, 1], f32, tag="gw_c")
            nc.gpsimd.dma_start(out=gw_c, in_=gw_d[:, :])
            gb_c = const.tile([C, 1], f32, tag="gb_c")
            nc.gpsimd.dma_start(out=gb_c, in_=gb_d[:, :])

            prev_xn_inst = None
            for grp_lo, grp_n in ((0, 1), (1, 1), (2, 2), (4, 4)):
                # ---- phase A: groupnorm stats for the whole group ----
                GRPn = grp_n
                x_ts = []
                grp_all = gnp.tile([32, 8 * GRP], f32, tag="grp_all")
                for j in range(GRPn):
                    b = grp_lo + j
                    x_t = xin.tile([C, HW], f32, tag="x")
                    if b < 2:
                        # split first loads across both HWDGE queues (ACT is
                        # idle at startup) to halve time-to-first-stats
                        nc.sync.dma_start(out=x_t[0:64, :], in_=x_d[b, 0:64, :])
                        nc.scalar.dma_start(out=x_t[64:128, :], in_=x_d[b, 64:128, :])
                    else:
                        nc.sync.dma_start(out=x_t, in_=x_d[b, :, :])
                    x_ts.append(x_t)

                    stats = small.tile([C, 2, 6], f32, tag="stats")
                    bn0 = nc.vector.bn_stats(out=stats[:, 0, :], in_=x_t[:, 0:512])
                    if j == 0 and prev_xn_inst is not None:
                        _add_dep_helper(bn0.ins, prev_xn_inst.ins, sync=False,
                                        reason="order gn after prev group xn")
                    nc.vector.bn_stats(out=stats[:, 1, :], in_=x_t[:, 512:1024])
                    mv = small.tile([C, 2], f32, tag="mv")
                    nc.vector.bn_aggr(out=mv, in_=stats)

                    # stk = [mean_c, E2_c]  (E2 = var + mean^2)
                    stk = small.tile([C, 2], f32, tag="stk")
                    nc.vector.tensor_copy(out=stk[:, 0:1], in_=mv[:, 0:1])
                    tmp1 = small.tile([C, 1], f32, tag="tmp1")
                    nc.vector.tensor_mul(out=tmp1, in0=mv[:, 0:1], in1=mv[:, 0:1])
                    nc.vector.tensor_add(out=stk[:, 1:2], in0=mv[:, 1:2], in1=tmp1)

                    if grp_lo == 0:
                        stk_r = small.tile([C, 2], f32r, tag="stk_r")
                        nc.vector.tensor_copy(out=stk_r, in_=stk)
                        stk_r0 = stk_r
                    else:
                        # [128,2] -> [32,8]: row g = (m,E2) of its 4 channels
                        nc.gpsimd.dma_start(out=grp_all[:, 8 * j:8 * (j + 1)], in_=stk)

                if grp_lo == 0:
                    # PE-based combine for lowest-latency startup:
                    # [mean_g, E2_g] = G^T stk ; broadcast back via R^T
                    gn0 = ps_o2.tile([32, 2], f32, tag="o2")
                    nc.tensor.matmul(gn0, gmat_s, stk_r0, start=True, stop=True)
                    gsb2 = gnp.tile([32, 2], f32, tag="gsb2")
                    e2e = gnp.tile([32, 1], f32, tag="e2e")
                    nc.vector.tensor_scalar(
                        out=e2e, in0=gn0[:, 1:2], scalar1=EPS, scalar2=None, op0=OP.add)
                    nc.vector.tensor_copy(out=gsb2[:, 0:1], in_=gn0[:, 0:1])
                    m20 = gnp.tile([32, 1], f32, tag="m20")
                    nc.vector.tensor_mul(out=m20, in0=gsb2[:, 0:1], in1=gsb2[:, 0:1])
                    v0 = gnp.tile([32, 1], f32, tag="v0")
                    nc.vector.tensor_sub(out=v0, in0=e2e, in1=m20)
                    sq0 = gnp.tile([32, 1], f32, tag="sq0")
                    nc.scalar.activation(out=sq0, in_=v0, func=AF.Sqrt, bias=0.0, scale=1.0)
                    nc.vector.reciprocal(out=gsb2[:, 1:2], in_=sq0)
                    gsb2r = gnp.tile([32, 2], f32r, tag="gsb2r")
                    nc.vector.tensor_copy(out=gsb2r, in_=gsb2)
                    bc0 = ps_o2.tile([C, 2], f32, tag="o2")
                    nc.tensor.matmul(bc0, rmat_s, gsb2r, start=True, stop=True)
                    bc = gnp.tile([C, 2 * GRP], f32, tag="bc")
                    nc.vector.tensor_copy(out=bc[:, 0:2], in_=bc0)
                else:
                    # s12[g, b, t] = sum_r grp_all[g, 8b+2r+t]
                    s12 = gnp.tile([32, GRP, 2], f32, tag="s12")
                    nc.vector.reduce_sum(
                        out=s12[:, :GRPn, :],
                        in_=grp_all[:, :8 * GRPn].rearrange(
                            "g (b r t) -> g b t r", b=GRPn, t=2),
                        axis=AX.X,
                    )
                    # gsb layout [32, (b t)]: col 2j = mean_g, col 2j+1 = rstd_g
                    gsb = gnp.tile([32, 2 * GRP], f32, tag="gsb")
                    gsb_bt = gsb.rearrange("g (b t) -> g t b", t=2)
                    mean_v = gsb_bt[:, 0, :GRPn]      # [32, GRPn] strided
                    nc.vector.tensor_scalar_mul(out=mean_v, in0=s12[:, :GRPn, 0], scalar1=0.25)
                    e2g = gnp.tile([32, GRP], f32, tag="e2g")   # 0.25*s2 + eps
                    nc.vector.tensor_scalar(
                        out=e2g[:, :GRPn], in0=s12[:, :GRPn, 1], scalar1=0.25, scalar2=EPS,
                        op0=OP.mult, op1=OP.add,
                    )
                    m2g = gnp.tile([32, GRP], f32, tag="m2g")
                    nc.vector.tensor_mul(out=m2g[:, :GRPn], in0=mean_v, in1=mean_v)
                    varg = gnp.tile([32, GRP], f32, tag="varg")  # var + eps
                    nc.vector.tensor_sub(out=varg[:, :GRPn], in0=e2g[:, :GRPn], in1=m2g[:, :GRPn])
                    sqg = gnp.tile([32, GRP], f32, tag="sqg")
                    nc.scalar.activation(out=sqg[:, :GRPn], in_=varg[:, :GRPn],
                                         func=AF.Sqrt, bias=0.0, scale=1.0)
                    nc.vector.reciprocal(out=gsb_bt[:, 1, :GRPn], in_=sqg[:, :GRPn])

                    # broadcast group stats: [32, 2G] -> [128, 2G] (per 4 channels)
                    bc = gnp.tile([C, 2 * GRP], f32, tag="bc")
                    gsb_sub = gsb[:, :2 * GRPn]
                    gsb_rep = bass.AP(
                        tensor=gsb_sub.tensor, offset=gsb_sub.offset,
                        ap=[list(gsb_sub.ap[0]), [0, 4], list(gsb_sub.ap[1])],
                    )
                    nc.gpsimd.dma_start(out=bc[:, :2 * GRPn], in_=gsb_rep)

                # ---- phase B: per-batch attention ----
                for j in range(GRPn):
                    b = grp_lo + j
                    x_t = x_ts[j]

                    # scl = rstd*gn_w ; sh = gn_b - mean*scl
                    scl = small.tile([C, 1], f32, tag="scl")
                    nc.vector.tensor_mul(out=scl, in0=bc[:, 2 * j + 1:2 * j + 2], in1=gw_c)
                    tmp2 = small.tile([C, 1], f32, tag="tmp2")
                    nc.vector.tensor_mul(out=tmp2, in0=bc[:, 2 * j:2 * j + 1], in1=scl)
                    sh = small.tile([C, 1], f32, tag="sh")
                    nc.vector.tensor_sub(out=sh, in0=gb_c, in1=tmp2)

                    xn = xnp.tile([C, HW], f32r, tag="xn")
                    xn_inst = nc.vector.tensor_scalar(
                        out=xn, in0=x_t, scalar1=scl, scalar2=sh,
                        op0=OP.mult, op1=OP.add,
                    )
                    prev_xn_inst = xn_inst

                    # xb = x + bo_eff (residual + bias, off the critical tail)
                    xb = epi.tile([C, HW], f32, tag="xb")
                    xb_inst = nc.vector.tensor_scalar(
                        out=xb, in0=x_t, scalar1=bo_c, scalar2=None, op0=OP.add,
                    )
                    _add_dep_helper(xb_inst.ins, xn_inst.ins, sync=False,
                                    reason="xb after xn")

                    # ---- q/k/W projections ----
                    qT_ps = ps_att.tile([C, HW], f32, tag="att")
                    nc.tensor.matmul(qT_ps[:, 0:512], wq_r, xn[:, 0:512], start=True, stop=True)
                    nc.tensor.matmul(qT_ps[:, 512:1024], wq_r, xn[:, 512:1024], start=True, stop=True)
                    qT = qkw.tile([C, HW], f32r, tag="qT")
                    nc.vector.tensor_scalar(
                        out=qT, in0=qT_ps, scalar1=bq_c, scalar2=None, op0=OP.add,
                    )

                    kT_ps = ps_att.tile([C, HW], f32, tag="att")
                    nc.tensor.matmul(kT_ps[:, 0:512], wk_r, xn[:, 0:512], start=True, stop=True)
                    nc.tensor.matmul(kT_ps[:, 512:1024], wk_r, xn[:, 512:1024], start=True, stop=True)
                    kT = qkw.tile([C, HW], f32r, tag="kT")
                    nc.scalar.activation(out=kT, in_=kT_ps, func=AF.Identity, bias=bk_c, scale=1.0)

                    # W[t, c'] = sum_c xn[c, t] * wvo_t[c, c']
                    W_ps = ps_att.tile([C, HW], f32, tag="att")
                    for blk in range(NBLK):
                        nc.tensor.matmul(
                            W_ps[:, blk * 128:(blk + 1) * 128],
                            xn[:, blk * 128:(blk + 1) * 128], wvo_r,
                            start=True, stop=True,
                        )
                    W_sb = qkw.tile([C, HW], bf16, tag="W_sb")
                    nc.vector.tensor_copy(out=W_sb, in_=W_ps)

                    # ---- attention ----
                    row_ps = ps_row.tile([C, HW], f32, tag="row")
                    o2_ps = ps_o2.tile([C, HW], f32, tag="o2")
                    exs = []
                    def _row_o2(i):
                        first, last = i == 0, i == NBLK - 1
                        exi = exs[i]
                        nc.tensor.matmul(row_ps[:, 0:512], ones_r, exi[:, 0:512], start=first, stop=last)
                        nc.tensor.matmul(row_ps[:, 512:1024], ones_r, exi[:, 512:1024], start=first, stop=last)
                        wblk = W_sb[:, i * 128:(i + 1) * 128]
                        nc.tensor.matmul(o2_ps[:, 0:512], wblk, exi[:, 0:512], start=first, stop=last)
                        nc.tensor.matmul(o2_ps[:, 512:1024], wblk, exi[:, 512:1024], start=first, stop=last)

                    for blk in range(NBLK):
                        attT = ps_att.tile([C, HW], f32, tag="att")
                        kblk = kT[:, blk * 128:(blk + 1) * 128]
                        nc.tensor.matmul(attT[:, 0:512], kblk, qT[:, 0:512], start=True, stop=True)
                        nc.tensor.matmul(attT[:, 512:1024], kblk, qT[:, 512:1024], start=True, stop=True)
                        ex = expp.tile([C, HW], bf16, tag="ex")
                        nc.scalar.activation(out=ex, in_=attT, func=AF.Exp, scale=SCALE)
                        exs.append(ex)
                        if blk >= 1:
                            _row_o2(blk - 1)
                    _row_o2(NBLK - 1)

                    # ---- epilogue: normalize + (residual+bias) ----
                    recip = epi.tile([C, HW], f32, tag="recip")
                    nc.vector.reciprocal_approx_fast(out=recip, in_=row_ps)
                    t3 = epi.tile([C, HW], f32, tag="t3")
                    nc.vector.tensor_mul(out=t3, in0=o2_ps, in1=recip)
                    out_t = epi.tile([C, HW], f32, tag="out_t")
                    nc.vector.tensor_add(out=out_t, in0=t3, in1=xb)
                    nc.sync.dma_start(out=out_d[b, :, :], in_=out_t)

    nc.finalize()
    return nc


def _get_nc():
    global _NC_CACHE
    if _NC_CACHE is None:
        _NC_CACHE = _build_nc()
    return _NC_CACHE


def _make_in_maps(x, gn_w, gn_b, wq, bq, wk, bk, wv, bv, wo, bo):
    x = np.ascontiguousarray(np.asarray(x, dtype=np.float32))
    xr = x.reshape(B, C, HW)
    wq64, wk64 = np.float64(wq), np.float64(wk)
    wv64, wo64 = np.float64(wv), np.float64(wo)
    wvo = wo64 @ wv64
    bo_eff = (np.float64(bo) + wo64 @ np.float64(bv)).astype(np.float32)
    gmat = np.zeros((C, 32), np.float32)
    rmat = np.zeros((32, C), np.float32)
    for c in range(C):
        gmat[c, c // 4] = 0.25
        rmat[c // 4, c] = 1.0
    common = {
        "gmat": gmat,
        "rmat": rmat,
        "wq_t": np.ascontiguousarray(wq64.T.astype(np.float32)),
        "wk_t": np.ascontiguousarray(wk64.T.astype(np.float32)),
        "wvo_t": np.ascontiguousarray(wvo.T.astype(np.float32)),
        "bq": np.asarray(bq, np.float32).reshape(C, 1),
        "bk": np.asarray(bk, np.float32).reshape(C, 1),
        "bo_eff": bo_eff.reshape(C, 1),
        "gn_w": np.asarray(gn_w, np.float32).reshape(C, 1),
        "gn_b": np.asarray(gn_b, np.float32).reshape(C, 1),
    }
    return [
        {"x": np.ascontiguousarray(xr[i * BPC:(i + 1) * BPC]), **common}
        for i in range(N_CORES)
    ]


def kernel(x, gn_w, gn_b, wq, bq, wk, bk, wv, bv, wo, bo):
    in_maps = _make_in_maps(x, gn_w, gn_b, wq, bq, wk, bk, wv, bv, wo, bo)
    nc = _get_nc()
    res = run_bass_kernel_spmd(nc, in_maps, list(range(N_CORES)))
    out = np.concatenate([res.results[i]["out"] for i in range(N_CORES)], axis=0)
    return out.reshape(B, C, 32, 32)


# revision 25
# speedup vs baseline: 1.1058x; 1.1058x over previous
"""GroupNorm + single-head self-attention + residual block on 8 trn2 cores.

Reference computation (per batch item b of 64):
    xn = GroupNorm32(x[b]) * gn_w + gn_b          # x[b]: [C=128, HW=1024]
    t  = xn^T                                     # [S=1024, C=128]
    q, k, v = t@wq^T+bq, t@wk^T+bk, t@wv^T+bv
    att = softmax(q k^T / sqrt(512))
    out[b] = (att v) @ wo^T + bo  (as [C, HW])  + x[b]

Sharding: pure data parallel, 8 batch items per core, params replicated.

Kernel layout choices (per batch item, all on-chip):
  - channels live on SBUF partitions; sequence S=1024 on the free dim
  - attention scores computed TRANSPOSED: attT[t, s] = kT^T qT, so the
    P^T needed by the output matmul is produced directly and no PE
    transposes are needed
  - softmax skips the max-subtraction (logits are provably in [-2, 2]);
    exp via ScalarE; row sums via an all-ones stationary matmul that
    accumulates exp blocks into a partition-broadcast PSUM tile
  - wv and wo are fused on the host: W = xn^T @ (wo@wv)^T, so attention
    output directly accumulates o2^T = W^T exp without a second
    projection; v-bias folds into bo_eff = bo + wo@bv; normalization by
    the softmax sum commutes to the very end
  - matmul operands are bf16 (outputs accumulate in fp32 PSUM); the
    residual path, groupnorm stats and softmax normalization stay fp32
  - groupnorm stats are hoisted per 4-batch group (one Sqrt table load
    per group instead of per batch)
"""

import numpy as np

import concourse.bacc as bacc
import concourse.bass as bass
import concourse.tile as tile
from concourse import mybir
from concourse.bass import _add_dep_helper
from concourse.bass_utils import run_bass_kernel_spmd

f32 = mybir.dt.float32
f32r = mybir.dt.float32r
bf16 = mybir.dt.bfloat16
AX = mybir.AxisListType
AF = mybir.ActivationFunctionType
OP = mybir.AluOpType

N_CORES = 8
B, C, HW = 64, 128, 1024
BPC = B // N_CORES          # batch items per core
NBLK = HW // 128            # 8 key blocks of 128
GRP = 4                     # batches per groupnorm stats group
SCALE = 0.044194173824159216
EPS = 1e-6

_NC_CACHE = None


def _build_nc():
    nc = bacc.Bacc()

    x_d = nc.declare_dram_parameter("x", [BPC, C, HW], f32, isOutput=False)
    wq_d = nc.declare_dram_parameter("wq_t", [C, C], f32, isOutput=False)
    wk_d = nc.declare_dram_parameter("wk_t", [C, C], f32, isOutput=False)
    wvo_d = nc.declare_dram_parameter("wvo_t", [C, C], f32, isOutput=False)
    bq_d = nc.declare_dram_parameter("bq", [C, 1], f32, isOutput=False)
    bk_d = nc.declare_dram_parameter("bk", [C, 1], f32, isOutput=False)
    bo_d = nc.declare_dram_parameter("bo_eff", [C, 1], f32, isOutput=False)
    gw_d = nc.declare_dram_parameter("gn_w", [C, 1], f32, isOutput=False)
    gb_d = nc.declare_dram_parameter("gn_b", [C, 1], f32, isOutput=False)
    gmat_d = nc.declare_dram_parameter("gmat", [C, 32], f32r, isOutput=False)
    rmat_d = nc.declare_dram_parameter("rmat", [32, C], f32r, isOutput=False)
    out_d = nc.declare_dram_parameter("out", [BPC, C, HW], f32, isOutput=True)

    with tile.TileContext(nc) as tc:
        with (
            tc.tile_pool(name="const", bufs=1) as const,
            tc.tile_pool(name="xin", bufs=8) as xin,
            tc.tile_pool(name="xnp", bufs=2) as xnp,
            tc.tile_pool(name="qkw", bufs=3) as qkw,
            tc.tile_pool(name="expp", bufs=5) as expp,
            tc.tile_pool(name="epi", bufs=2) as epi,
            tc.tile_pool(name="small", bufs=4) as small,
            tc.tile_pool(name="gn", bufs=2) as gnp,
            tc.tile_pool(name="ps_att", bufs=2, space="PSUM") as ps_att,
            tc.tile_pool(name="ps_row", bufs=1, space="PSUM") as ps_row,
            tc.tile_pool(name="ps_o2", bufs=1, space="PSUM") as ps_o2,
        ):
            # ---- one-time constants ----
            stage = const.tile([C, C], f32, tag="stage_q")
            nc.gpsimd.dma_start(out=stage, in_=wq_d[:, :])
            wq_r = const.tile([C, C], bf16, tag="wq_r")
            nc.gpsimd.tensor_copy(out=wq_r, in_=stage)

            stage2 = const.tile([C, C], f32, tag="stage_k")
            nc.gpsimd.dma_start(out=stage2, in_=wk_d[:, :])
            wk_r = const.tile([C, C], bf16, tag="wk_r")
            nc.gpsimd.tensor_copy(out=wk_r, in_=stage2)

            stage3 = const.tile([C, C], f32, tag="stage_v")
            nc.gpsimd.dma_start(out=stage3, in_=wvo_d[:, :])
            wvo_r = const.tile([C, C], bf16, tag="wvo_r")
            nc.gpsimd.tensor_copy(out=wvo_r, in_=stage3)

            ones_s = const.tile([C, C], f32, tag="ones_s")
            nc.vector.memset(ones_s, 1.0)
            ones_r = const.tile([C, C], bf16, tag="ones_r")
            nc.gpsimd.tensor_copy(out=ones_r, in_=ones_s)

            gmat_s = const.tile([C, 32], f32r, tag="gmat_s")
            nc.sync.dma_start(out=gmat_s, in_=gmat_d[:, :])
            rmat_s = const.tile([32, C], f32r, tag="rmat_s")
            nc.sync.dma_start(out=rmat_s, in_=rmat_d[:, :])

            bq_c = const.tile([C, 1], f32, tag="bq_c")
            nc.gpsimd.dma_start(out=bq_c, in_=bq_d[:, :])
            bk_c = const.tile([C, 1], f32, tag="bk_c")
            nc.gpsimd.dma_start(out=bk_c, in_=bk_d[:, :])
            bo_c = const.tile([C, 1], f32, tag="bo_c")
            nc.gpsimd.dma_start(out=bo_c, in_=bo_d[:, :])
            gw_c = const.tile([C, 1], f32, tag="gw_c")
            nc.gpsimd.dma_start(out=gw_c, in_=gw_d[:, :])
            gb_c = const.tile([C, 1], f32, tag="gb_c")
            nc.gpsimd.dma_start(out=gb_c, in_=gb_d[:, :])

            prev_xn_inst = None
            for grp_lo, grp_n in ((0, 1), (1, 1), (2, 2), (4, 4)):
                # ---- phase A: groupnorm stats for the whole group ----
                GRPn = grp_n
                x_ts = []
                grp_all = gnp.tile([32, 8 * GRP], f32, tag="grp_all")
                for j in range(GRPn):
                    b = grp_lo + j
                    x_t = xin.tile([C, HW], f32, tag="x")
                    if b < 2:
                        # split first loads across both HWDGE queues (ACT is
                        # idle at startup) to halve time-to-first-stats
                        nc.sync.dma_start(out=x_t[0:64, :], in_=x_d[b, 0:64, :])
                        nc.scalar.dma_start(out=x_t[64:128, :], in_=x_d[b, 64:128, :])
                    else:
                        nc.sync.dma_start(out=x_t, in_=x_d[b, :, :])
                    x_ts.append(x_t)

                    stats = small.tile([C, 2, 6], f32, tag="stats")
                    bn0 = nc.vector.bn_stats(out=stats[:, 0, :], in_=x_t[:, 0:512])
                    if j == 0 and prev_xn_inst is not None:
                        _add_dep_helper(bn0.ins, prev_xn_inst.ins, sync=False,
                                        reason="order gn after prev group xn")
                    nc.vector.bn_stats(out=stats[:, 1, :], in_=x_t[:, 512:1024])
                    mv = small.tile([C, 2], f32, tag="mv")
                    nc.vector.bn_aggr(out=mv, in_=stats)

                    # stk = [mean_c, E2_c]  (E2 = var + mean^2)
                    stk = small.tile([C, 2], f32, tag="stk")
                    nc.vector.tensor_copy(out=stk[:, 0:1], in_=mv[:, 0:1])
                    tmp1 = small.tile([C, 1], f32, tag="tmp1")
                    nc.vector.tensor_mul(out=tmp1, in0=mv[:, 0:1], in1=mv[:, 0:1])
                    nc.vector.tensor_add(out=stk[:, 1:2], in0=mv[:, 1:2], in1=tmp1)

                    if grp_lo == 0:
                        stk_r = small.tile([C, 2], f32r, tag="stk_r")
                        nc.vector.tensor_copy(out=stk_r, in_=stk)
                        stk_r0 = stk_r
                    else:
                        # [128,2] -> [32,8]: row g = (m,E2) of its 4 channels
                        nc.gpsimd.dma_start(out=grp_all[:, 8 * j:8 * (j + 1)], in_=stk)

                if grp_lo == 0:
                    # PE-based combine for lowest-latency startup:
                    # [mean_g, E2_g] = G^T stk ; broadcast back via R^T
                    gn0 = ps_o2.tile([32, 2], f32, tag="o2")
                    nc.tensor.matmul(gn0, gmat_s, stk_r0, start=True, stop=True)
                    gsb2 = gnp.tile([32, 2], f32, tag="gsb2")
                    e2e = gnp.tile([32, 1], f32, tag="e2e")
                    nc.vector.tensor_scalar(
                        out=e2e, in0=gn0[:, 1:2], scalar1=EPS, scalar2=None, op0=OP.add)
                    nc.vector.tensor_copy(out=gsb2[:, 0:1], in_=gn0[:, 0:1])
                    m20 = gnp.tile([32, 1], f32, tag="m20")
                    nc.vector.tensor_mul(out=m20, in0=gsb2[:, 0:1], in1=gsb2[:, 0:1])
                    v0 = gnp.tile([32, 1], f32, tag="v0")
                    nc.vector.tensor_sub(out=v0, in0=e2e, in1=m20)
                    sq0 = gnp.tile([32, 1], f32, tag="sq0")
                    nc.scalar.activation(out=sq0, in_=v0, func=AF.Sqrt, bias=0.0, scale=1.0)
                    nc.vector.reciprocal(out=gsb2[:, 1:2], in_=sq0)
                    gsb2r = gnp.tile([32, 2], f32r, tag="gsb2r")
                    nc.vector.tensor_copy(out=gsb2r, in_=gsb2)
                    bc0 = ps_o2.tile([C, 2], f32, tag="o2")
                    nc.tensor.matmul(bc0, rmat_s, gsb2r, start=True, stop=True)
                    bc = gnp.tile([C, 2 * GRP], f32, tag="bc")
                    nc.vector.tensor_copy(out=bc[:, 0:2], in_=bc0)
                else:
                    # s12[g, b, t] = sum_r grp_all[g, 8b+2r+t]
                    s12 = gnp.tile([32, GRP, 2], f32, tag="s12")
                    nc.vector.reduce_sum(
                        out=s12[:, :GRPn, :],
                        in_=grp_all[:, :8 * GRPn].rearrange(
                            "g (b r t) -> g b t r", b=GRPn, t=2),
                        axis=AX.X,
                    )
                    # gsb layout [32, (b t)]: col 2j = mean_g, col 2j+1 = rstd_g
                    gsb = gnp.tile([32, 2 * GRP], f32, tag="gsb")
                    gsb_bt = gsb.rearrange("g (b t) -> g t b", t=2)
                    mean_v = gsb_bt[:, 0, :GRPn]      # [32, GRPn] strided
                    nc.vector.tensor_scalar_mul(out=mean_v, in0=s12[:, :GRPn, 0], scalar1=0.25)
                    e2g = gnp.tile([32, GRP], f32, tag="e2g")   # 0.25*s2 + eps
                    nc.vector.tensor_scalar(
                        out=e2g[:, :GRPn], in0=s12[:, :GRPn, 1], scalar1=0.25, scalar2=EPS,
                        op0=OP.mult, op1=OP.add,
                    )
                    m2g = gnp.tile([32, GRP], f32, tag="m2g")
                    nc.vector.tensor_mul(out=m2g[:, :GRPn], in0=mean_v, in1=mean_v)
                    varg = gnp.tile([32, GRP], f32, tag="varg")  # var + eps
                    nc.vector.tensor_sub(out=varg[:, :GRPn], in0=e2g[:, :GRPn], in1=m2g[:, :GRPn])
                    sqg = gnp.tile([32, GRP], f32, tag="sqg")
                    nc.scalar.activation(out=sqg[:, :GRPn], in_=varg[:, :GRPn],
                                         func=AF.Sqrt, bias=0.0, scale=1.0)
                    nc.vector.reciprocal(out=gsb_bt[:, 1, :GRPn], in_=sqg[:, :GRPn])

                    # broadcast group stats: [32, 2G] -> [128, 2G] (per 4 channels)
                    bc = gnp.tile([C, 2 * GRP], f32, tag="bc")
                    gsb_sub = gsb[:, :2 * GRPn]
                    gsb_rep = bass.AP(
                        tensor=gsb_sub.tensor, offset=gsb_sub.offset,
                        ap=[list(gsb_sub.ap[0]), [0, 4], list(gsb_sub.ap[1])],
                    )
                    nc.gpsimd.dma_start(out=bc[:, :2 * GRPn], in_=gsb_rep)

                # ---- phase B: per-batch attention ----
                for j in range(GRPn):
                    b = grp_lo + j
                    x_t = x_ts[j]

                    # scl = rstd*gn_w ; sh = gn_b - mean*scl
                    scl = small.tile([C, 1], f32, tag="scl")
                    nc.vector.tensor_mul(out=scl, in0=bc[:, 2 * j + 1:2 * j + 2], in1=gw_c)
                    tmp2 = small.tile([C, 1], f32, tag="tmp2")
                    nc.vector.tensor_mul(out=tmp2, in0=bc[:, 2 * j:2 * j + 1], in1=scl)
                    sh = small.tile([C, 1], f32, tag="sh")
                    nc.vector.tensor_sub(out=sh, in0=gb_c, in1=tmp2)

                    xn = xnp.tile([C, HW], bf16, tag="xn")
                    xn_inst = nc.vector.tensor_scalar(
                        out=xn, in0=x_t, scalar1=scl, scalar2=sh,
                        op0=OP.mult, op1=OP.add,
                    )
                    prev_xn_inst = xn_inst

                    # xb = x + bo_eff (residual + bias, off the critical tail)
                    xb = epi.tile([C, HW], f32, tag="xb")
                    xb_inst = nc.vector.tensor_scalar(
                        out=xb, in0=x_t, scalar1=bo_c, scalar2=None, op0=OP.add,
                    )
                    _add_dep_helper(xb_inst.ins, xn_inst.ins, sync=False,
                                    reason="xb after xn")

                    # ---- q/k/W projections ----
                    qT_ps = ps_att.tile([C, HW], f32, tag="att")
                    nc.tensor.matmul(qT_ps[:, 0:512], wq_r, xn[:, 0:512], start=True, stop=True)
                    nc.tensor.matmul(qT_ps[:, 512:1024], wq_r, xn[:, 512:1024], start=True, stop=True)
                    qT = qkw.tile([C, HW], bf16, tag="qT")
                    nc.vector.tensor_scalar(
                        out=qT, in0=qT_ps, scalar1=bq_c, scalar2=None, op0=OP.add,
                    )

                    kT_ps = ps_att.tile([C, HW], f32, tag="att")
                    nc.tensor.matmul(kT_ps[:, 0:512], wk_r, xn[:, 0:512], start=True, stop=True)
                    nc.tensor.matmul(kT_ps[:, 512:1024], wk_r, xn[:, 512:1024], start=True, stop=True)
                    kT = qkw.tile([C, HW], bf16, tag="kT")
                    nc.scalar.activation(out=kT, in_=kT_ps, func=AF.Identity, bias=bk_c, scale=1.0)

                    # W[t, c'] = sum_c xn[c, t] * wvo_t[c, c']
                    W_ps = ps_att.tile([C, HW], f32, tag="att")
                    for blk in range(NBLK):
                        nc.tensor.matmul(
                            W_ps[:, blk * 128:(blk + 1) * 128],
                            xn[:, blk * 128:(blk + 1) * 128], wvo_r,
                            start=True, stop=True,
                        )
                    W_sb = qkw.tile([C, HW], bf16, tag="W_sb")
                    nc.vector.tensor_copy(out=W_sb, in_=W_ps)

                    # ---- attention ----
                    row_ps = ps_row.tile([C, HW], f32, tag="row")
                    o2_ps = ps_o2.tile([C, HW], f32, tag="o2")
                    exs = []
                    def _row_o2(i):
                        first, last = i == 0, i == NBLK - 1
                        exi = exs[i]
                        nc.tensor.matmul(row_ps[:, 0:512], ones_r, exi[:, 0:512], start=first, stop=last)
                        nc.tensor.matmul(row_ps[:, 512:1024], ones_r, exi[:, 512:1024], start=first, stop=last)
                        wblk = W_sb[:, i * 128:(i + 1) * 128]
                        nc.tensor.matmul(o2_ps[:, 0:512], wblk, exi[:, 0:512], start=first, stop=last)
                        nc.tensor.matmul(o2_ps[:, 512:1024], wblk, exi[:, 512:1024], start=first, stop=last)

                    for blk in range(NBLK):
                        attT = ps_att.tile([C, HW], f32, tag="att")
                        kblk = kT[:, blk * 128:(blk + 1) * 128]
                        nc.tensor.matmul(attT[:, 0:512], kblk, qT[:, 0:512], start=True, stop=True)
                        nc.tensor.matmul(attT[:, 512:1024], kblk, qT[:, 512:1024], start=True, stop=True)
                        ex = expp.tile([C, HW], bf16, tag="ex")
                        nc.scalar.activation(out=ex, in_=attT, func=AF.Exp, scale=SCALE)
                        exs.append(ex)
                        if blk >= 1:
                            _row_o2(blk - 1)
                    _row_o2(NBLK - 1)

                    # ---- epilogue: normalize + (residual+bias) ----
                    recip = epi.tile([C, HW], f32, tag="recip")
                    nc.vector.reciprocal_approx_fast(out=recip, in_=row_ps)
                    t3 = epi.tile([C, HW], f32, tag="t3")
                    nc.vector.tensor_mul(out=t3, in0=o2_ps, in1=recip)
                    out_t = epi.tile([C, HW], f32, tag="out_t")
                    nc.vector.tensor_add(out=out_t, in0=t3, in1=xb)
                    nc.sync.dma_start(out=out_d[b, :, :], in_=out_t)

    nc.finalize()
    return nc


def _get_nc():
    global _NC_CACHE
    if _NC_CACHE is None:
        _NC_CACHE = _build_nc()
    return _NC_CACHE


def _make_in_maps(x, gn_w, gn_b, wq, bq, wk, bk, wv, bv, wo, bo):
    x = np.ascontiguousarray(np.asarray(x, dtype=np.float32))
    xr = x.reshape(B, C, HW)
    wq64, wk64 = np.float64(wq), np.float64(wk)
    wv64, wo64 = np.float64(wv), np.float64(wo)
    wvo = wo64 @ wv64
    bo_eff = (np.float64(bo) + wo64 @ np.float64(bv)).astype(np.float32)
    gmat = np.zeros((C, 32), np.float32)
    rmat = np.zeros((32, C), np.float32)
    for c in range(C):
        gmat[c, c // 4] = 0.25
        rmat[c // 4, c] = 1.0
    common = {
        "gmat": gmat,
        "rmat": rmat,
        "wq_t": np.ascontiguousarray(wq64.T.astype(np.float32)),
        "wk_t": np.ascontiguousarray(wk64.T.astype(np.float32)),
        "wvo_t": np.ascontiguousarray(wvo.T.astype(np.float32)),
        "bq": np.asarray(bq, np.float32).reshape(C, 1),
        "bk": np.asarray(bk, np.float32).reshape(C, 1),
        "bo_eff": bo_eff.reshape(C, 1),
        "gn_w": np.asarray(gn_w, np.float32).reshape(C, 1),
        "gn_b": np.asarray(gn_b, np.float32).reshape(C, 1),
    }
    return [
        {"x": np.ascontiguousarray(xr[i * BPC:(i + 1) * BPC]), **common}
        for i in range(N_CORES)
    ]


def kernel(x, gn_w, gn_b, wq, bq, wk, bk, wv, bv, wo, bo):
    in_maps = _make_in_maps(x, gn_w, gn_b, wq, bq, wk, bk, wv, bv, wo, bo)
    nc = _get_nc()
    res = run_bass_kernel_spmd(nc, in_maps, list(range(N_CORES)))
    out = np.concatenate([res.results[i]["out"] for i in range(N_CORES)], axis=0)
    return out.reshape(B, C, 32, 32)
